# revision 2
# baseline (speedup 1.0000x reference)
"""Trainium2 Bass kernel v2 for nn_DifferentiableEmbedding (moe_routing).

Sorted-by-expert design. Per token: y = (emb[id] * mask) @ W[e].T + b[e] with
mask = (iota < 512*gate[id]) and e = clip(ceil(512*gate)//102, 0, 4).

Key points:
  * mask and e are pure functions of the vocab id -> host bakes a PREMASKED
    embedding table xm[v] = emb[v] * mask(v) (weights-only prep) plus a tiny
    e-table. Tokens of expert e only need the first chunks_e*128 features
    (chunks = [1,2,3,4,4]).
  * the device sorts tokens by expert (20 slot-tiles of 128, 4 per expert):
    e-gather (16 indirect DMAs) -> counting-rank via one DVE scan + one
    triangular-matmul partition prefix -> scatter [id, orig] into DRAM
    staging by slot (16 indirect DMAs) -> reload sorted.
  * each sorted tile runs ONLY its expert's chunks: 56 accumulating matmuls
    of 512 cols total (vs 224 unsorted) and no predicated selects at all.
  * bias rides the matmul for e<4 (xT last-chunk row 127 is always-zero ->
    overwritten with 1.0, matching wt row holds b[e]); e=4 adds a
    PE-broadcast bias tile on DVE.
  * y is written in sorted order plus an order vector; the host inverts the
    permutation while unsharding (pad slots carry sentinel order 3000).
"""

import os
import sys

import numpy as np

sys.path.insert(0, "/opt/trn_rl_repo")

import concourse.bass as bass  # noqa: E402
import concourse.tile as tile  # noqa: E402
from concourse import bacc, bass_utils, mybir  # noqa: E402

F32 = mybir.dt.float32
F32R = mybir.dt.float32r
I32 = mybir.dt.int32

VOCAB, D, B, S, E = 50257, 512, 8, 2048, 5
P = 128
NT = S // P                    # 16 original token tiles
CHUNKS = [1, 2, 3, 4, 4]
NJ = sum(CHUNKS)               # 14 weight chunk-columns
TPE = 4                        # slot tiles per expert
NST = TPE * E                  # 20 sorted tiles
NSLOT = 21 * P                 # staging rows (>= NST*P)
EW = 16                        # e-table row width (f32) = 64B
SENT = 3000.0                  # pad sentinel for order output


def build_program():
    nc = bacc.Bacc("TRN2", target_bir_lowering=False, debug=False,
                   enable_asserts=False, num_devices=8)

    ids_f = nc.dram_tensor("ids_f", [P, NT], F32, kind="ExternalInput").ap()
    etab = nc.dram_tensor("etab", [VOCAB, EW], F32, kind="ExternalInput").ap()
    xm_half = nc.dram_tensor("xm_half", [VOCAB, 256], F32, kind="ExternalInput").ap()
    xm_full = nc.dram_tensor("xm_full", [VOCAB, D], F32, kind="ExternalInput").ap()
    wt = nc.dram_tensor("wt", [P, NJ, D], F32R, kind="ExternalInput").ap()
    b4 = nc.dram_tensor("b4", [1, D], F32, kind="ExternalInput").ap()

    ys = nc.dram_tensor("ys", [NST * P, D], F32, kind="ExternalOutput").ap()
    order = nc.dram_tensor("order", [P, NST], F32, kind="ExternalOutput").ap()

    # compile-time consts embedded in the NEFF
    epat_np = np.broadcast_to(
        np.arange(E, dtype=np.float32)[None, :, None], (P, E, NT)).copy()
    reset_np = np.ones((P, E, NT), np.float32)
    reset_np[:, :, 0] = 0.0
    eoffm1_np = np.broadcast_to(
        (np.arange(E, dtype=np.float32) * (TPE * P) - 1.0)[None, :], (P, E)).copy()
    tri_np = np.triu(np.ones((P, P), np.float32), 1)  # [q, p] = 1 if q < p
    ident_np = np.eye(P, dtype=np.float32)
    ones_np = np.ones((1, P), np.float32)
    init_np = np.zeros((P, 4), np.float32)
    init_np[:, 1] = SENT
    orig_np = (np.arange(NT, dtype=np.float32)[None, :] * P
               + np.arange(P, dtype=np.float32)[:, None])

    epat_t = nc.inline_tensor(epat_np.reshape(P, E * NT), name="epat").ap()
    reset_t = nc.inline_tensor(reset_np.reshape(P, E * NT), name="reset").ap()
    eoffm1_t = nc.inline_tensor(eoffm1_np, name="eoffm1").ap()
    tri_t = nc.inline_tensor(tri_np, name="tri").ap()
    ident_t = nc.inline_tensor(ident_np, name="ident").ap()
    ones_t = nc.inline_tensor(ones_np, name="ones1").ap()
    init_t = nc.inline_tensor(init_np, name="init4").ap()
    orig_t = nc.inline_tensor(orig_np, name="orig").ap()

    with tile.TileContext(nc) as tc:
        with (
            tc.tile_pool(name="singles", bufs=1) as sg,
            tc.tile_pool(name="gpool", bufs=1) as gp,
            tc.tile_pool(name="xt", bufs=1) as xt,
            tc.tile_pool(name="yout", bufs=1) as yo,
            tc.tile_pool(name="tp_ps", bufs=1, space="PSUM") as tp_ps,
            tc.tile_pool(name="y_ps", bufs=1, space="PSUM") as y_ps,
            tc.tile_pool(name="m_ps", bufs=1, space="PSUM") as m_ps,
            tc.tile_pool(name="dr", bufs=1, space="DRAM") as dr,
        ):
            # ---- ids first on the scalar queue ----
            idsf_sb = sg.tile([P, NT], F32)
            nc.scalar.dma_start(out=idsf_sb[:], in_=ids_f[:, :])
            ids_i = sg.tile([P, NT], I32)
            nc.scalar.activation(out=ids_i[:], in_=idsf_sb[:],
                                 func=mybir.ActivationFunctionType.Copy)

            # ---- constants / weights on sync queue (overlap e-gather) ----
            epat_sb = sg.tile([P, E, NT], F32)
            nc.sync.dma_start(out=epat_sb[:],
                              in_=epat_t[:, :].rearrange("p (e t) -> p e t", t=NT))
            reset_sb = sg.tile([P, E * NT], F32)
            nc.sync.dma_start(out=reset_sb[:], in_=reset_t[:, :])
            eoffm1_sb = sg.tile([P, E], F32)
            nc.sync.dma_start(out=eoffm1_sb[:], in_=eoffm1_t[:, :])
            tri_sb = sg.tile([P, P], F32)
            nc.sync.dma_start(out=tri_sb[:], in_=tri_t[:, :])
            ident_sb = sg.tile([P, P], F32R)
            nc.sync.dma_start(out=ident_sb[:], in_=ident_t[:, :].bitcast(F32R))
            ones_sb = sg.tile([1, P], F32)
            nc.sync.dma_start(out=ones_sb[:], in_=ones_t[:, :])
            b4_sb = sg.tile([1, D], F32)
            nc.sync.dma_start(out=b4_sb[:], in_=b4[:, :])
            init_sb = sg.tile([P, 4], F32)
            nc.sync.dma_start(out=init_sb[:], in_=init_t[:, :])
            orig_sb = sg.tile([P, NT], F32)
            nc.sync.dma_start(out=orig_sb[:], in_=orig_t[:, :])
            wt_sb = sg.tile([P, NJ, D], F32R)
            for j in range(NJ):
                nc.sync.dma_start(out=wt_sb[:, j, :], in_=wt[:, j, :])

            # Bcast4 = ones^T (x) b4 via 1-partition outer-product matmul
            bc_ps = m_ps.tile([P, D], F32, tag="bcps")
            nc.tensor.matmul(out=bc_ps[:], lhsT=ones_sb[:], rhs=b4_sb[:],
                             start=True, stop=True)
            bcast4 = sg.tile([P, D], F32)
            nc.scalar.activation(out=bcast4[:], in_=bc_ps[:],
                                 func=mybir.ActivationFunctionType.Copy)

            # ---- e-gather: expert id per token (16 indirect DMAs) ----
            eout = sg.tile([P, NT, EW], F32)
            for t in range(NT):
                nc.gpsimd.indirect_dma_start(
                    out=eout[:, t, :], out_offset=None,
                    in_=etab[:, :],
                    in_offset=bass.IndirectOffsetOnAxis(
                        ap=ids_i[:, t:t + 1], axis=0),
                )

            # ---- sort math ----
            e_col = eout[:, :, 0]                      # [P, NT] strided
            m_sb = sg.tile([P, E, NT], F32)
            nc.vector.tensor_tensor(
                out=m_sb[:],
                in0=e_col.unsqueeze(1).to_broadcast([P, E, NT]),
                in1=epat_sb[:], op=mybir.AluOpType.is_equal)
            c_sb = sg.tile([P, E, NT], F32)
            nc.vector.tensor_tensor_scan(
                out=c_sb[:].rearrange("p e t -> p (e t)"),
                data0=reset_sb[:],
                data1=m_sb[:].rearrange("p e t -> p (e t)"),
                initial=0.0, op0=mybir.AluOpType.mult, op1=mybir.AluOpType.add)
            poff_ps = m_ps.tile([P, E], F32, tag="poff")
            nc.tensor.matmul(out=poff_ps[:], lhsT=tri_sb[:],
                             rhs=c_sb[:, :, NT - 1], start=True, stop=True)
            p2s = sg.tile([P, E], F32)
            nc.vector.tensor_tensor(out=p2s[:], in0=poff_ps[:],
                                    in1=eoffm1_sb[:], op=mybir.AluOpType.add)
            a_sb = sg.tile([P, E, NT], F32)
            nc.vector.tensor_tensor(
                out=a_sb[:], in0=c_sb[:],
                in1=p2s[:].unsqueeze(2).to_broadcast([P, E, NT]),
                op=mybir.AluOpType.add)
            nc.vector.tensor_tensor(out=a_sb[:], in0=a_sb[:], in1=m_sb[:],
                                    op=mybir.AluOpType.mult)
            pos_f = sg.tile([P, NT], F32)
            nc.vector.tensor_reduce(
                out=pos_f[:], in_=a_sb[:].rearrange("p e t -> p t e"),
                axis=mybir.AxisListType.X, op=mybir.AluOpType.add)
            pos_i = sg.tile([P, NT], I32)
            nc.scalar.activation(out=pos_i[:], in_=pos_f[:],
                                 func=mybir.ActivationFunctionType.Copy)

            # ---- scatter [id, orig] to staging, reload sorted ----
            staging = dr.tile([NSLOT, 4], F32)
            nc.sync.dma_start(
                out=staging[:, :].rearrange("(j p) c -> p j c", p=P),
                in_=init_sb[:].unsqueeze(1).to_broadcast([P, NSLOT // P, 4]))
            pay = sg.tile([P, NT, 4], F32)
            nc.vector.memset(pay[:], 0.0)
            nc.vector.tensor_copy(pay[:, :, 0], idsf_sb[:])
            nc.vector.tensor_copy(pay[:, :, 1], orig_sb[:])
            # orig[p, t] = t*128 + p: build from iota consts? host const via
            # epat-like inline: reuse scan trick is overkill -> inline tensor
            for t in range(NT):
                nc.gpsimd.indirect_dma_start(
                    out=staging[:, :],
                    out_offset=bass.IndirectOffsetOnAxis(
                        ap=pos_i[:, t:t + 1], axis=0),
                    in_=pay[:, t, :], in_offset=None,
                    bounds_check=NSLOT - 1, oob_is_err=False,
                )
            srt = sg.tile([P, NST, 4], F32)
            nc.sync.dma_start(
                out=srt[:],
                in_=staging[:, :].rearrange("(j p) c -> p j c", p=P)[:, 0:NST, :])
            sid_i = sg.tile([P, NST], I32)
            nc.scalar.activation(out=sid_i[:], in_=srt[:, :, 0],
                                 func=mybir.ActivationFunctionType.Copy)
            nc.sync.dma_start(out=order[:, :], in_=srt[:, :, 1])

            # ---- sorted per-tile compute ----
            for j in range(NST):
                e = j // TPE
                ce = CHUNKS[e]
                w_cols = 256 if e <= 1 else D
                src = xm_half if e <= 1 else xm_full
                g_full = gp.tile([P, D], F32R, tag=f"g{j % 6}")
                g_j = g_full[:, 0:w_cols]
                nc.gpsimd.indirect_dma_start(
                    out=g_j[:], out_offset=None, in_=src[:, :],
                    in_offset=bass.IndirectOffsetOnAxis(
                        ap=sid_i[:, j:j + 1], axis=0),
                )
                wdat = ce * P
                tp_full = tp_ps.tile([P, 4 * P], F32R, tag=f"tp{j % 2}")
                tp = tp_full[:, 0:wdat]
                for k in range(ce):
                    nc.tensor.matmul(
                        out=tp[:, k * P:(k + 1) * P],
                        lhsT=g_j[:, k * P:(k + 1) * P],
                        rhs=ident_sb[:], is_transpose=True,
                        start=(k == 0), stop=(k == ce - 1))
                xT_full = xt.tile([P, 4 * P], F32R, tag=f"x{j % 6}")
                xT = xT_full[:, 0:wdat]
                nc.scalar.activation(
                    out=xT[:], in_=tp[:],
                    func=mybir.ActivationFunctionType.Copy)
                bank = y_ps.tile([P, D], F32, tag=f"b{j % 4}")
                for k in range(ce):
                    jw = sum(CHUNKS[:e]) + k
                    nc.tensor.matmul(
                        out=bank[:], lhsT=xT[:, k * P:(k + 1) * P],
                        rhs=wt_sb[:, jw, :],
                        start=(k == 0), stop=(k == ce - 1))
                y_sb = yo.tile([P, D], F32, tag=f"y{j % 4}")
                if e < 4:
                    nc.scalar.activation(
                        out=y_sb[:], in_=bank[:],
                        func=mybir.ActivationFunctionType.Copy)
                else:
                    nc.vector.tensor_tensor(
                        out=y_sb[:], in0=bank[:], in1=bcast4[:],
                        op=mybir.AluOpType.add)
                nc.sync.dma_start(out=ys[j * P:(j + 1) * P, :], in_=y_sb[:])

    nc.compile()
    return nc


def prep_tables(emb_table, gate_table, expert_w, expert_b):
    """Weights-only host prep."""
    g = gate_table[:, 0].astype(np.float32) * np.float32(D)
    count = np.ceil(g)
    eidx = np.clip((count // float(D // E)).astype(np.int64), 0, E - 1)
    iota = np.arange(D, dtype=np.float32)
    mask = (iota[None, :] < g[:, None]).astype(np.float32)
    xm_full = np.ascontiguousarray(emb_table * mask, dtype=np.float32)
    # bake the bias-carrier 1.0 into the always-zero feature 128*(e+1)-1
    xm_full[eidx == 2, 383] = 1.0
    xm_full[eidx == 3, 511] = 1.0
    xm_half = np.ascontiguousarray(xm_full[:, 0:256])
    xm_half[eidx == 0, 127] = 1.0
    xm_half[eidx == 1, 255] = 1.0
    etab = np.zeros((VOCAB, EW), np.float32)
    etab[:, 0] = eidx.astype(np.float32)
    # wt[p, (e, k), o] = W[e][o, 128k+p]; stolen bias row for e<4
    wt_full = np.transpose(expert_w, (2, 0, 1)).reshape(D // P, P, E, D)
    cols = []
    for e in range(E):
        for k in range(CHUNKS[e]):
            cols.append(wt_full[k, :, e, :])
    wt = np.ascontiguousarray(np.stack(cols, axis=1), dtype=np.float32)
    for e in range(4):
        j = sum(CHUNKS[:e]) + CHUNKS[e] - 1
        wt[P - 1, j, :] = expert_b[e]
    b4 = expert_b[4:5].astype(np.float32)
    return {"etab": etab, "xm_half": xm_half, "xm_full": xm_full,
            "wt": wt, "b4": b4}


_CACHED_NC = None


def kernel(input_ids, emb_table, gate_table, expert_w, expert_b):
    global _CACHED_NC
    input_ids = np.asarray(input_ids)
    emb_table = np.asarray(emb_table, dtype=np.float32)
    gate_table = np.asarray(gate_table, dtype=np.float32)
    expert_w = np.asarray(expert_w, dtype=np.float32)
    expert_b = np.asarray(expert_b, dtype=np.float32)

    if _CACHED_NC is None:
        _CACHED_NC = build_program()
    nc = _CACHED_NC

    shared = prep_tables(emb_table, gate_table, expert_w, expert_b)
    in_maps = []
    for c in range(B):
        ids_pt = np.ascontiguousarray(
            input_ids[c].reshape(NT, P).T.astype(np.float32))  # [P, NT]
        m = dict(shared)
        m["ids_f"] = ids_pt
        in_maps.append(m)

    trace = bool(int(os.environ.get("BASS_KERNEL_TRACE", "0")))
    res = bass_utils.run_bass_kernel_spmd(
        nc, in_maps, core_ids=list(range(B)), trace=trace)
    kernel.last_result = res

    out = np.empty((B, S, D), np.float32)
    for c in range(B):
        ys = res.results[c]["ys"]                    # [NST*P, D]
        order = res.results[c]["order"]              # [P, NST]
        ord_flat = np.transpose(order).reshape(-1).astype(np.int64)  # slot->orig
        valid = ord_flat < S
        out[c, ord_flat[valid]] = ys[valid]
    return out.astype(np.float32)


kernel.last_result = None


# revision 3
# speedup vs baseline: 1.0273x; 1.0273x over previous
"""Trainium2 Bass kernel v2 for nn_DifferentiableEmbedding (moe_routing).

Sorted-by-expert design. Per token: y = (emb[id] * mask) @ W[e].T + b[e] with
mask = (iota < 512*gate[id]) and e = clip(ceil(512*gate)//102, 0, 4).

Key points:
  * mask and e are pure functions of the vocab id -> host bakes a PREMASKED
    embedding table xm[v] = emb[v] * mask(v) (weights-only prep) plus a tiny
    e-table. Tokens of expert e only need the first chunks_e*128 features
    (chunks = [1,2,3,4,4]).
  * the device sorts tokens by expert (20 slot-tiles of 128, 4 per expert):
    e-gather (16 indirect DMAs) -> counting-rank via one DVE scan + one
    triangular-matmul partition prefix -> scatter [id, orig] into DRAM
    staging by slot (16 indirect DMAs) -> reload sorted.
  * each sorted tile runs ONLY its expert's chunks: 56 accumulating matmuls
    of 512 cols total (vs 224 unsorted) and no predicated selects at all.
  * bias rides the matmul for e<4 (xT last-chunk row 127 is always-zero ->
    overwritten with 1.0, matching wt row holds b[e]); e=4 adds a
    PE-broadcast bias tile on DVE.
  * y is written in sorted order plus an order vector; the host inverts the
    permutation while unsharding (pad slots carry sentinel order 3000).
"""

import os
import sys

import numpy as np

sys.path.insert(0, "/opt/trn_rl_repo")

import concourse.bass as bass  # noqa: E402
import concourse.tile as tile  # noqa: E402
from concourse import bacc, bass_utils, mybir  # noqa: E402

F32 = mybir.dt.float32
F32R = mybir.dt.float32r
I32 = mybir.dt.int32

VOCAB, D, B, S, E = 50257, 512, 8, 2048, 5
P = 128
NT = S // P                    # 16 original token tiles
CHUNKS = [1, 2, 3, 4, 4]
NJ = sum(CHUNKS)               # 14 weight chunk-columns
TPE = 4                        # slot tiles per expert
NST = TPE * E                  # 20 sorted tiles
NSLOT = 21 * P                 # staging rows (>= NST*P)
EW = 16                        # e-table row width (f32) = 64B
SENT = 3000.0                  # pad sentinel for order output


def build_program():
    nc = bacc.Bacc("TRN2", target_bir_lowering=False, debug=False,
                   enable_asserts=False, num_devices=8)

    ids_f = nc.dram_tensor("ids_f", [P, NT], F32, kind="ExternalInput").ap()
    ids_32 = nc.dram_tensor("ids_32", [P, NT], I32, kind="ExternalInput").ap()
    etab = nc.dram_tensor("etab", [VOCAB, EW], F32, kind="ExternalInput").ap()
    xm_half = nc.dram_tensor("xm_half", [VOCAB, 256], F32, kind="ExternalInput").ap()
    xm_full = nc.dram_tensor("xm_full", [VOCAB, D], F32, kind="ExternalInput").ap()
    wt = nc.dram_tensor("wt", [P, NJ, D], F32R, kind="ExternalInput").ap()
    b4 = nc.dram_tensor("b4", [1, D], F32, kind="ExternalInput").ap()

    staging = nc.dram_tensor("staging", [NSLOT, 4], F32, kind="Internal").ap()
    ys = nc.dram_tensor("ys", [NST * P, D], F32, kind="ExternalOutput").ap()
    order = nc.dram_tensor("order", [P, NST], F32, kind="ExternalOutput").ap()

    # compile-time consts embedded in the NEFF
    epat_np = np.broadcast_to(
        np.arange(E, dtype=np.float32)[None, :, None], (P, E, NT)).copy()
    reset_np = np.ones((P, E, NT), np.float32)
    reset_np[:, :, 0] = 0.0
    eoffm1_np = np.broadcast_to(
        (np.arange(E, dtype=np.float32) * (TPE * P) - 1.0)[None, :], (P, E)).copy()
    tri_np = np.triu(np.ones((P, P), np.float32), 1)  # [q, p] = 1 if q < p
    ident_np = np.eye(P, dtype=np.float32)
    ones_np = np.ones((1, P), np.float32)
    init_np = np.zeros((P, 4), np.float32)
    init_np[:, 1] = SENT
    orig_np = (np.arange(NT, dtype=np.float32)[None, :] * P
               + np.arange(P, dtype=np.float32)[:, None])

    epat_t = nc.inline_tensor(epat_np.reshape(P, E * NT), name="epat").ap()
    reset_t = nc.inline_tensor(reset_np.reshape(P, E * NT), name="reset").ap()
    eoffm1_t = nc.inline_tensor(eoffm1_np, name="eoffm1").ap()
    tri_t = nc.inline_tensor(tri_np, name="tri").ap()
    ident_t = nc.inline_tensor(ident_np, name="ident").ap()
    ones_t = nc.inline_tensor(ones_np, name="ones1").ap()
    init_t = nc.inline_tensor(init_np, name="init4").ap()
    orig_t = nc.inline_tensor(orig_np, name="orig").ap()

    with tile.TileContext(nc) as tc:
        with (
            tc.tile_pool(name="singles", bufs=1) as sg,
            tc.tile_pool(name="gpool", bufs=1) as gp,
            tc.tile_pool(name="xt", bufs=1) as xt,
            tc.tile_pool(name="yout", bufs=1) as yo,
            tc.tile_pool(name="tp_ps", bufs=1, space="PSUM") as tp_ps,
            tc.tile_pool(name="y_ps", bufs=1, space="PSUM") as y_ps,
            tc.tile_pool(name="m_ps", bufs=1, space="PSUM") as m_ps,
        ):
            # ---- ids first on the scalar queue ----
            ids_i = sg.tile([P, NT], I32)
            nc.scalar.dma_start(out=ids_i[:], in_=ids_32[:, :])
            idsf_sb = sg.tile([P, NT], F32)
            nc.scalar.dma_start(out=idsf_sb[:], in_=ids_f[:, :])

            # ---- constants / weights on sync queue (overlap e-gather) ----
            epat_sb = sg.tile([P, E, NT], F32)
            nc.sync.dma_start(out=epat_sb[:],
                              in_=epat_t[:, :].rearrange("p (e t) -> p e t", t=NT))
            reset_sb = sg.tile([P, E * NT], F32)
            nc.sync.dma_start(out=reset_sb[:], in_=reset_t[:, :])
            eoffm1_sb = sg.tile([P, E], F32)
            nc.sync.dma_start(out=eoffm1_sb[:], in_=eoffm1_t[:, :])
            tri_sb = sg.tile([P, P], F32)
            nc.sync.dma_start(out=tri_sb[:], in_=tri_t[:, :])
            ident_sb = sg.tile([P, P], F32R)
            nc.sync.dma_start(out=ident_sb[:], in_=ident_t[:, :].bitcast(F32R))
            ones_sb = sg.tile([1, P], F32)
            nc.sync.dma_start(out=ones_sb[:], in_=ones_t[:, :])
            b4_sb = sg.tile([1, D], F32)
            nc.sync.dma_start(out=b4_sb[:], in_=b4[:, :])
            init_sb = sg.tile([P, 4], F32)
            nc.sync.dma_start(out=init_sb[:], in_=init_t[:, :])
            orig_sb = sg.tile([P, NT], F32)
            nc.sync.dma_start(out=orig_sb[:], in_=orig_t[:, :])
            wt_sb = sg.tile([P, NJ, D], F32R)
            for j in range(NJ):
                nc.sync.dma_start(out=wt_sb[:, j, :], in_=wt[:, j, :])

            # Bcast4 = ones^T (x) b4 via 1-partition outer-product matmul
            bc_ps = m_ps.tile([P, D], F32, tag="bcps")
            nc.tensor.matmul(out=bc_ps[:], lhsT=ones_sb[:], rhs=b4_sb[:],
                             start=True, stop=True)
            bcast4 = sg.tile([P, D], F32)
            nc.scalar.activation(out=bcast4[:], in_=bc_ps[:],
                                 func=mybir.ActivationFunctionType.Copy)

            # ---- e-gather: expert id per token (16 indirect DMAs) ----
            eout = sg.tile([P, NT, EW], F32)
            for t in range(NT):
                nc.gpsimd.indirect_dma_start(
                    out=eout[:, t, :], out_offset=None,
                    in_=etab[:, :],
                    in_offset=bass.IndirectOffsetOnAxis(
                        ap=ids_i[:, t:t + 1], axis=0),
                )

            # ---- sort math ----
            e_col = eout[:, :, 0]                      # [P, NT] strided
            m_sb = sg.tile([P, E, NT], F32)
            nc.vector.tensor_tensor(
                out=m_sb[:],
                in0=e_col.unsqueeze(1).to_broadcast([P, E, NT]),
                in1=epat_sb[:], op=mybir.AluOpType.is_equal)
            c_sb = sg.tile([P, E, NT], F32)
            nc.vector.tensor_tensor_scan(
                out=c_sb[:].rearrange("p e t -> p (e t)"),
                data0=reset_sb[:],
                data1=m_sb[:].rearrange("p e t -> p (e t)"),
                initial=0.0, op0=mybir.AluOpType.mult, op1=mybir.AluOpType.add)
            poff_ps = m_ps.tile([P, E], F32, tag="poff")
            nc.tensor.matmul(out=poff_ps[:], lhsT=tri_sb[:],
                             rhs=c_sb[:, :, NT - 1], start=True, stop=True)
            p2s = sg.tile([P, E], F32)
            nc.vector.tensor_tensor(out=p2s[:], in0=poff_ps[:],
                                    in1=eoffm1_sb[:], op=mybir.AluOpType.add)
            a_sb = sg.tile([P, E, NT], F32)
            nc.vector.tensor_tensor(
                out=a_sb[:], in0=c_sb[:],
                in1=p2s[:].unsqueeze(2).to_broadcast([P, E, NT]),
                op=mybir.AluOpType.add)
            nc.vector.tensor_tensor(out=a_sb[:], in0=a_sb[:], in1=m_sb[:],
                                    op=mybir.AluOpType.mult)
            pos_f = sg.tile([P, NT], F32)
            nc.vector.tensor_reduce(
                out=pos_f[:], in_=a_sb[:].rearrange("p e t -> p t e"),
                axis=mybir.AxisListType.X, op=mybir.AluOpType.add)
            pos_i = sg.tile([P, NT], I32)
            nc.scalar.activation(out=pos_i[:], in_=pos_f[:],
                                 func=mybir.ActivationFunctionType.Copy)

            # ---- scatter [id, orig] to staging, reload sorted ----
            sem_init = nc.alloc_semaphore(name="sem_init")
            sem_scat = nc.alloc_semaphore(name="sem_scat")
            nc.sync.dma_start(
                out=staging[:, :].rearrange("(j p) c -> p j c", p=P),
                in_=init_sb[:].unsqueeze(1).to_broadcast([P, NSLOT // P, 4]))
            nc.sync.drain()
            nc.sync.sem_inc(sem_init, 1)
            nc.gpsimd.wait_ge(sem_init, 1)
            pay = sg.tile([P, NT, 4], F32)
            nc.vector.memset(pay[:], 0.0)
            nc.vector.tensor_copy(pay[:, :, 0], idsf_sb[:])
            nc.vector.tensor_copy(pay[:, :, 1], orig_sb[:])
            # orig[p, t] = t*128 + p: build from iota consts? host const via
            # epat-like inline: reuse scan trick is overkill -> inline tensor
            for t in range(NT):
                nc.gpsimd.indirect_dma_start(
                    out=staging[:, :],
                    out_offset=bass.IndirectOffsetOnAxis(
                        ap=pos_i[:, t:t + 1], axis=0),
                    in_=pay[:, t, :], in_offset=None,
                    bounds_check=NSLOT - 1, oob_is_err=False,
                )
            nc.gpsimd.drain()
            nc.gpsimd.sem_inc(sem_scat, 1)
            nc.sync.wait_ge(sem_scat, 1)
            srt = sg.tile([P, NST, 4], F32)
            nc.sync.dma_start(
                out=srt[:],
                in_=staging[:, :].rearrange("(j p) c -> p j c", p=P)[:, 0:NST, :])
            sid_i = sg.tile([P, NST], I32)
            nc.scalar.activation(out=sid_i[:], in_=srt[:, :, 0],
                                 func=mybir.ActivationFunctionType.Copy)
            nc.sync.dma_start(out=order[:, :], in_=srt[:, :, 1])

            # ---- sorted per-tile compute ----
            for j in range(NST):
                e = j // TPE
                ce = CHUNKS[e]
                w_cols = 256 if e <= 1 else D
                src = xm_half if e <= 1 else xm_full
                g_full = gp.tile([P, D], F32R, tag=f"g{j % 6}")
                g_j = g_full[:, 0:w_cols]
                nc.gpsimd.indirect_dma_start(
                    out=g_j[:], out_offset=None, in_=src[:, :],
                    in_offset=bass.IndirectOffsetOnAxis(
                        ap=sid_i[:, j:j + 1], axis=0),
                )
                wdat = ce * P
                tp_full = tp_ps.tile([P, 4 * P], F32R, tag=f"tp{j % 2}")
                tp = tp_full[:, 0:wdat]
                for k in range(ce):
                    nc.tensor.matmul(
                        out=tp[:, k * P:(k + 1) * P],
                        lhsT=g_j[:, k * P:(k + 1) * P],
                        rhs=ident_sb[:], is_transpose=True,
                        start=(k == 0), stop=(k == ce - 1))
                xT_full = xt.tile([P, 4 * P], F32R, tag=f"x{j % 6}")
                xT = xT_full[:, 0:wdat]
                nc.scalar.activation(
                    out=xT[:], in_=tp[:],
                    func=mybir.ActivationFunctionType.Copy)
                bank = y_ps.tile([P, D], F32, tag=f"b{j % 4}")
                for k in range(ce):
                    jw = sum(CHUNKS[:e]) + k
                    nc.tensor.matmul(
                        out=bank[:], lhsT=xT[:, k * P:(k + 1) * P],
                        rhs=wt_sb[:, jw, :],
                        start=(k == 0), stop=(k == ce - 1))
                y_sb = yo.tile([P, D], F32, tag=f"y{j % 4}")
                if e < 4:
                    nc.scalar.activation(
                        out=y_sb[:], in_=bank[:],
                        func=mybir.ActivationFunctionType.Copy)
                else:
                    nc.vector.tensor_tensor(
                        out=y_sb[:], in0=bank[:], in1=bcast4[:],
                        op=mybir.AluOpType.add)
                nc.sync.dma_start(out=ys[j * P:(j + 1) * P, :], in_=y_sb[:])

    nc.compile()
    return nc


def prep_tables(emb_table, gate_table, expert_w, expert_b):
    """Weights-only host prep."""
    g = gate_table[:, 0].astype(np.float32) * np.float32(D)
    count = np.ceil(g)
    eidx = np.clip((count // float(D // E)).astype(np.int64), 0, E - 1)
    iota = np.arange(D, dtype=np.float32)
    mask = (iota[None, :] < g[:, None]).astype(np.float32)
    xm_full = np.ascontiguousarray(emb_table * mask, dtype=np.float32)
    # bake the bias-carrier 1.0 into the always-zero feature 128*(e+1)-1
    xm_full[eidx == 2, 383] = 1.0
    xm_full[eidx == 3, 511] = 1.0
    xm_half = np.ascontiguousarray(xm_full[:, 0:256])
    xm_half[eidx == 0, 127] = 1.0
    xm_half[eidx == 1, 255] = 1.0
    etab = np.zeros((VOCAB, EW), np.float32)
    etab[:, 0] = eidx.astype(np.float32)
    # wt[p, (e, k), o] = W[e][o, 128k+p]; stolen bias row for e<4
    wt_full = np.transpose(expert_w, (2, 0, 1)).reshape(D // P, P, E, D)
    cols = []
    for e in range(E):
        for k in range(CHUNKS[e]):
            cols.append(wt_full[k, :, e, :])
    wt = np.ascontiguousarray(np.stack(cols, axis=1), dtype=np.float32)
    for e in range(4):
        j = sum(CHUNKS[:e]) + CHUNKS[e] - 1
        wt[P - 1, j, :] = expert_b[e]
    b4 = expert_b[4:5].astype(np.float32)
    return {"etab": etab, "xm_half": xm_half, "xm_full": xm_full,
            "wt": wt, "b4": b4}


_CACHED_NC = None


def kernel(input_ids, emb_table, gate_table, expert_w, expert_b):
    global _CACHED_NC
    input_ids = np.asarray(input_ids)
    emb_table = np.asarray(emb_table, dtype=np.float32)
    gate_table = np.asarray(gate_table, dtype=np.float32)
    expert_w = np.asarray(expert_w, dtype=np.float32)
    expert_b = np.asarray(expert_b, dtype=np.float32)

    if _CACHED_NC is None:
        _CACHED_NC = build_program()
    nc = _CACHED_NC

    shared = prep_tables(emb_table, gate_table, expert_w, expert_b)
    in_maps = []
    for c in range(B):
        ids_pt = np.ascontiguousarray(input_ids[c].reshape(NT, P).T)  # [P, NT]
        m = dict(shared)
        m["ids_f"] = ids_pt.astype(np.float32)
        m["ids_32"] = ids_pt.astype(np.int32)
        in_maps.append(m)

    trace = bool(int(os.environ.get("BASS_KERNEL_TRACE", "0")))
    res = bass_utils.run_bass_kernel_spmd(
        nc, in_maps, core_ids=list(range(B)), trace=trace)
    kernel.last_result = res

    out = np.empty((B, S, D), np.float32)
    for c in range(B):
        ys = res.results[c]["ys"]                    # [NST*P, D]
        order = res.results[c]["order"]              # [P, NST]
        ord_flat = np.transpose(order).reshape(-1).astype(np.int64)  # slot->orig
        valid = ord_flat < S
        out[c, ord_flat[valid]] = ys[valid]
    return out.astype(np.float32)


kernel.last_result = None


# revision 4
# speedup vs baseline: 1.3082x; 1.2733x over previous
"""Trainium2 Bass kernel for nn_DifferentiableEmbedding (moe_routing).

Computation (per token t):
    data = emb_table[id]                      # (512,)
    g    = gate_table[id] * 512               # scalar in (0.512, 512)
    mask = (iota512 < g)                      # 0/1 mask (frac term is exactly 0 in f32)
    e    = clip(ceil(g) // 102, 0, 4)         # expert index
    y    = (data*mask) @ W[e].T + b[e]

Sharding: data-parallel on B (8 batch rows -> 8 cores). Tables and expert
weights replicated per core.

Key design points:
  * count = sum(mask) = ceil(g) exactly in f32 (the straight-through frac term
    rounds to exactly 0), so the expert index and selected bias row are pure
    functions of the vocab id.  e(v) and expert_b[e(v)] are therefore
    precomputed on the host from gate_table/expert_b (weights-only prep) and
    appended to each embedding-table row; one indirect gather per 128-token
    tile fetches [emb | gate | e(v) | bias-row] together.  (HW indirect DMA
    honors only one index per partition, so gathers are per-tile.)
  * tokens of expert e have mask zero beyond feature 102e+101, so expert e
    only needs the first ceil((102e+101)/128) of the 4 K-chunks: [1,2,3,4,4]
    -> 14 accumulating matmuls per 128-token tile instead of 20.
  * xm is transposed once per tile (4 PE transposes into one PSUM bank,
    PSUM->SBUF casts on the scalar engine); the 14 matmuls write 5 per-expert
    PSUM banks and the output rows are assembled with one ACT scale-copy +
    4 DVE predicated copies selected by the expert indicators, plus the
    gathered bias row.
  * matmuls run as float32r (full PE rate at N=512).
"""

import os
import sys

import numpy as np

sys.path.insert(0, "/opt/trn_rl_repo")

import concourse.bass as bass  # noqa: E402
import concourse.tile as tile  # noqa: E402
from concourse import bacc, bass_utils, mybir  # noqa: E402

VOCAB, D, B, S, E = 50257, 512, 8, 2048, 5
P = 128                     # partitions / tokens per tile
NT = S // P                 # 16 token tiles per core
NK = D // P                 # 4 contraction chunks
CHUNKS_PER_EXPERT = [1, 2, 3, 4, 4]   # tail-chunk trick
NJ = sum(CHUNKS_PER_EXPERT)           # 14 (expert, chunk) pairs

F32 = mybir.dt.float32
F32R = mybir.dt.float32r
I32 = mybir.dt.int32
I8 = mybir.dt.int8
# augmented row: [0:512] emb, [512] gate, [513] e(v), [514:528] pad,
# [528:1040] bias row of e(v)  -> 1040 f32 = 4160 B (64B-aligned)
DA = 1040
DG = 512   # gate column
DE = 513   # expert-index column
DB = 528   # bias row start
NH = NT // 2  # tiles per indicator half


def build_program(debug_taps=False):
    """Build the single-core Tile program (same program runs SPMD on 8 cores)."""
    nc = bacc.Bacc(
        "TRN2",
        target_bir_lowering=False,
        debug=False,
        enable_asserts=False,
        num_devices=8,
    )

    ids = nc.dram_tensor("ids", [P, NT], I32, kind="ExternalInput").ap()
    emb = nc.dram_tensor("emb", [VOCAB, DA], F32, kind="ExternalInput").ap()
    wt = nc.dram_tensor("wt", [P, NJ, D], F32R, kind="ExternalInput").ap()
    iota = nc.dram_tensor("iota", [P, D], F32, kind="ExternalInput").ap()
    ident = nc.dram_tensor("ident", [P, P], F32R, kind="ExternalInput").ap()
    iota5 = nc.dram_tensor("iota5", [P, E], F32, kind="ExternalInput").ap()
    y = nc.dram_tensor("y", [S, D], F32, kind="ExternalOutput").ap()
    if debug_taps:
        dbg_emb = nc.dram_tensor("dbg_emb", [P, NT, DA], F32, kind="ExternalOutput").ap()
        dbg_gsc = nc.dram_tensor("dbg_gsc", [P, NT], F32, kind="ExternalOutput").ap()
        dbg_ind = nc.dram_tensor("dbg_ind", [P, NT, E], F32, kind="ExternalOutput").ap()
        dbg_xm = nc.dram_tensor("dbg_xm", [P, D], F32, kind="ExternalOutput").ap()
        dbg_xmt = nc.dram_tensor("dbg_xmt", [P, P], F32R, kind="ExternalOutput").ap()

    with tile.TileContext(nc) as tc:
        with (
            tc.tile_pool(name="singles", bufs=1) as singles,
            tc.tile_pool(name="work", bufs=4) as work,
            tc.tile_pool(name="xmt", bufs=12) as xmt,
            tc.tile_pool(name="gpool", bufs=1) as gpool,
            tc.tile_pool(name="tp_ps", bufs=2, space="PSUM") as tp_ps,
            tc.tile_pool(name="y_ps", bufs=1, space="PSUM") as y_ps,
        ):
            # ids go first, on the scalar-engine HWDGE queue, so the gathers
            # are not stuck behind the weight DMAs on the sync queue
            ids_sb = singles.tile([P, NT], I32)
            nc.scalar.dma_start(out=ids_sb[:], in_=ids[:, :])

            # ---- constants (sync queue, overlaps the gathers) ----
            iota_sb = singles.tile([P, D], F32)
            nc.sync.dma_start(out=iota_sb[:], in_=iota[:, :])
            ident_sb = singles.tile([P, P], F32R)
            nc.sync.dma_start(out=ident_sb[:], in_=ident[:, :])
            iota5_sb = singles.tile([P, E], F32)
            nc.sync.dma_start(out=iota5_sb[:], in_=iota5[:, :])
            wt_sb = singles.tile([P, NJ, D], F32R)
            for j in range(NJ):
                nc.sync.dma_start(out=wt_sb[:, j, :], in_=wt[:, j, :])

            # gather [emb | gate | e(v) | bias] rows per 128-token tile; one
            # SBUF tile per gather so downstream deps are exact
            embs = []
            for t in range(NT):
                emb_t = gpool.tile([P, DA], F32, tag=f"emb{t}")
                nc.gpsimd.indirect_dma_start(
                    out=emb_t[:],
                    out_offset=None,
                    in_=emb[:, :],
                    in_offset=bass.IndirectOffsetOnAxis(
                        ap=ids_sb[:, t : t + 1], axis=0
                    ),
                )
                embs.append(emb_t)

            if debug_taps:
                for t in range(NT):
                    nc.sync.dma_start(out=dbg_emb[:, t, :], in_=embs[t][:])

            # ---- per 128-token tile ----
            for t in range(NT):
                emb_t = embs[t]
                # g = gate*512 (must round exactly like the reference)
                gsc_t = work.tile([P, 1], F32, tag="gsc")
                nc.vector.tensor_scalar(
                    out=gsc_t[:], in0=emb_t[:, DG : DG + 1], scalar1=float(D),
                    scalar2=None, op0=mybir.AluOpType.mult,
                )
                # one-hot expert indicators from the precomputed e(v) column
                ind_f = work.tile([P, E], F32, tag="indf")
                nc.vector.tensor_scalar(
                    out=ind_f[:], in0=iota5_sb[:], scalar1=emb_t[:, DE : DE + 1],
                    scalar2=None, op0=mybir.AluOpType.is_equal,
                )
                ind_i8 = work.tile([P, E], I8, tag="indi")
                nc.scalar.activation(
                    out=ind_i8[:], in_=ind_f[:],
                    func=mybir.ActivationFunctionType.Copy,
                )
                if debug_taps:
                    nc.sync.dma_start(out=dbg_gsc[:, t : t + 1], in_=gsc_t[:])
                    nc.sync.dma_start(out=dbg_ind[:, t, :], in_=ind_f[:])

                mask = work.tile([P, D], F32, tag="mask")
                nc.vector.tensor_scalar(
                    out=mask[:], in0=iota_sb[:], scalar1=gsc_t[:],
                    scalar2=None, op0=mybir.AluOpType.is_lt,
                )
                xm = work.tile([P, D], F32R, tag="xm")
                nc.vector.tensor_tensor(
                    out=xm[:], in0=mask[:], in1=emb_t[:, :D],
                    op=mybir.AluOpType.mult,
                )
                if debug_taps and t == 0:
                    nc.sync.dma_start(out=dbg_xm[:, :], in_=xm[:])

                # transpose the 4 K-chunks of xm into one PSUM bank; separate
                # xT tiles per chunk (cast split across DVE/ACT)
                tp = tp_ps.tile([P, 4 * P], F32R, tag="tp")
                for k in range(NK):
                    nc.tensor.matmul(
                        out=tp[:, k * P : (k + 1) * P],
                        lhsT=xm[:, k * P : (k + 1) * P],
                        rhs=ident_sb[:],
                        is_transpose=True,
                        start=(k == 0), stop=(k == NK - 1),
                    )
                xTs = []
                for k in range(NK):
                    xT_k = xmt.tile([P, P], F32R, tag=f"xmT{k}")
                    nc.scalar.activation(
                        out=xT_k[:], in_=tp[:, k * P : (k + 1) * P],
                        func=mybir.ActivationFunctionType.Copy,
                    )
                    xTs.append(xT_k)
                if debug_taps and t == 0:
                    nc.sync.dma_start(out=dbg_xmt[:, :], in_=xTs[0][:])

                # one PSUM bank per expert; expert e needs chunks 0..ce-1
                banks = []
                for e in range(E):
                    ybank = y_ps.tile([P, D], F32, tag=f"yps{e}")
                    banks.append(ybank)
                for e in range(E):
                    ce = CHUNKS_PER_EXPERT[e]
                    for k in range(ce):
                        j = sum(CHUNKS_PER_EXPERT[:e]) + k
                        nc.tensor.matmul(
                            out=banks[e][:],
                            lhsT=xTs[k][:],
                            rhs=wt_sb[:, j, :],
                            start=(k == 0), stop=(k == ce - 1),
                        )
                # assemble: rows of expert e from bank e, then add bias row
                y_sb = work.tile([P, D], F32, tag="ysb")
                nc.scalar.activation(
                    out=y_sb[:], in_=banks[0][:],
                    func=mybir.ActivationFunctionType.Copy,
                    scale=ind_f[:, 0:1],
                )
                for e in range(1, E):
                    nc.vector.copy_predicated(
                        out=y_sb[:],
                        mask=ind_i8[:, e : e + 1].to_broadcast([P, D]),
                        data=banks[e][:],
                    )
                nc.vector.tensor_tensor(
                    out=y_sb[:], in0=y_sb[:], in1=emb_t[:, DB:],
                    op=mybir.AluOpType.add,
                )
                nc.sync.dma_start(out=y[t * P : (t + 1) * P, :], in_=y_sb[:])

    nc.compile()
    return nc


def prep_core_inputs(input_ids_row, emb_table, gate_table, expert_w, expert_b,
                     aug=None):
    """Host-side layout prep for one core. input_ids_row: (S,) int."""
    ids = np.ascontiguousarray(
        input_ids_row.reshape(NT, P).T.astype(np.int32)
    )  # [P, NT]: ids[p, t] = token t*128+p
    if aug is None:
        aug = build_aug_table(emb_table, gate_table, expert_b)
    # wt[p, j, :] = expert_w[e].T[128k+p, :] = expert_w[e][:, 128k+p] for j=(e,k)
    wt_full = np.transpose(expert_w, (2, 0, 1)).reshape(NK, P, E, D)  # [k,p,e,o]
    cols = []
    for e in range(E):
        for k in range(CHUNKS_PER_EXPERT[e]):
            cols.append(wt_full[k, :, e, :])  # [P, D]
    wt = np.ascontiguousarray(np.stack(cols, axis=1), dtype=np.float32)  # [P,NJ,D]
    iota = np.ascontiguousarray(
        np.broadcast_to(np.arange(D, dtype=np.float32), (P, D))
    )
    ident = np.eye(P, dtype=np.float32)
    iota5 = np.ascontiguousarray(
        np.broadcast_to(np.arange(E, dtype=np.float32), (P, E))
    )
    return {
        "ids": ids,
        "emb": aug,
        "wt": wt,
        "iota": iota,
        "ident": ident,
        "iota5": iota5,
    }


def build_aug_table(emb_table, gate_table, expert_b):
    """Weights-only preprocessing: per vocab row v append gate, expert index
    e(v) = clip(ceil(gate*512)//102, 0, 4), and the selected bias row."""
    g = gate_table[:, 0].astype(np.float32) * np.float32(D)
    count = np.ceil(g)
    eidx = np.clip((count // float(D // E)).astype(np.int64), 0, E - 1)
    aug = np.zeros((VOCAB, DA), dtype=np.float32)
    aug[:, :D] = emb_table
    aug[:, DG] = gate_table[:, 0]
    aug[:, DE] = eidx.astype(np.float32)
    aug[:, DB:] = expert_b[eidx]
    return aug


_CACHED_NC = None


def kernel(input_ids, emb_table, gate_table, expert_w, expert_b):
    global _CACHED_NC
    input_ids = np.asarray(input_ids)
    emb_table = np.asarray(emb_table, dtype=np.float32)
    gate_table = np.asarray(gate_table, dtype=np.float32)
    expert_w = np.asarray(expert_w, dtype=np.float32)
    expert_b = np.asarray(expert_b, dtype=np.float32)

    if _CACHED_NC is None:
        _CACHED_NC = build_program()
    nc = _CACHED_NC

    shared = None
    in_maps = []
    for c in range(B):
        m = prep_core_inputs(
            input_ids[c], emb_table, gate_table, expert_w, expert_b,
            aug=None if shared is None else shared["emb"],
        )
        if shared is None:
            shared = m
        else:
            # reuse identical replicated arrays across cores
            for k_ in ("emb", "wt", "iota", "ident", "iota5"):
                m[k_] = shared[k_]
        in_maps.append(m)

    trace = bool(int(os.environ.get("BASS_KERNEL_TRACE", "0")))
    res = bass_utils.run_bass_kernel_spmd(
        nc, in_maps, core_ids=list(range(B)), trace=trace
    )
    kernel.last_result = res
    out = np.stack([res.results[c]["y"] for c in range(B)], axis=0)
    return out.astype(np.float32)



# revision 5
# speedup vs baseline: 1.6347x; 1.2496x over previous
"""Trainium2 Bass kernel for nn_DifferentiableEmbedding (moe_routing).

Computation (per token t):
    data = emb_table[id]                      # (512,)
    g    = gate_table[id] * 512               # scalar in (0.512, 512)
    mask = (iota512 < g)                      # 0/1 mask (frac term is exactly 0 in f32)
    e    = clip(ceil(g) // 102, 0, 4)         # expert index
    y    = (data*mask) @ W[e].T + b[e]

Sharding: data-parallel on B (8 batch rows -> 8 cores). Tables and expert
weights replicated per core.

Key design points:
  * count = sum(mask) = ceil(g) exactly in f32 (the straight-through frac term
    rounds to exactly 0), so the expert index and selected bias row are pure
    functions of the vocab id.  e(v) and expert_b[e(v)] are therefore
    precomputed on the host from gate_table/expert_b (weights-only prep) and
    appended to each embedding-table row; one indirect gather per 128-token
    tile fetches [emb | gate | e(v) | bias-row] together.  (HW indirect DMA
    honors only one index per partition, so gathers are per-tile.)
  * tokens of expert e have mask zero beyond feature 102e+101, so expert e
    only needs the first ceil((102e+101)/128) of the 4 K-chunks: [1,2,3,4,4]
    -> 14 accumulating matmuls per 128-token tile instead of 20.
  * xm is transposed once per tile (4 PE transposes into one PSUM bank,
    PSUM->SBUF casts on the scalar engine); the 14 matmuls write 5 per-expert
    PSUM banks and the output rows are assembled with one ACT scale-copy +
    4 DVE predicated copies selected by the expert indicators, plus the
    gathered bias row.
  * matmuls run as float32r (full PE rate at N=512).
"""

import os
import sys

import numpy as np

sys.path.insert(0, "/opt/trn_rl_repo")

import concourse.bass as bass  # noqa: E402
import concourse.tile as tile  # noqa: E402
from concourse import bacc, bass_utils, mybir  # noqa: E402

VOCAB, D, B, S, E = 50257, 512, 8, 2048, 5
P = 128                     # partitions / tokens per tile
NT = S // P                 # 16 token tiles per core
NK = D // P                 # 4 contraction chunks
CHUNKS_PER_EXPERT = [1, 2, 3, 4, 4]   # tail-chunk trick
NJ = sum(CHUNKS_PER_EXPERT)           # 14 (expert, chunk) pairs

F32 = mybir.dt.float32
F32R = mybir.dt.float32r
I32 = mybir.dt.int32
I8 = mybir.dt.int8
# augmented row: [0:512] premasked emb (with bias-carrier 1.0 for e<4),
# [512] e(v), [513:528] pad -> 528 f32 = 2112 B (64B-aligned)
DA = 528
DE = 512   # expert-index column
NH = NT // 2  # tiles per indicator half


def build_program(debug_taps=False):
    """Build the single-core Tile program (same program runs SPMD on 8 cores)."""
    nc = bacc.Bacc(
        "TRN2",
        target_bir_lowering=False,
        debug=False,
        enable_asserts=False,
        num_devices=8,
    )

    ids = nc.dram_tensor("ids", [P, NT], I32, kind="ExternalInput").ap()
    emb = nc.dram_tensor("emb", [VOCAB, DA], F32, kind="ExternalInput").ap()
    wt = nc.dram_tensor("wt", [P, NJ, D], F32R, kind="ExternalInput").ap()
    ident = nc.dram_tensor("ident", [P, P], F32R, kind="ExternalInput").ap()
    ones1 = nc.dram_tensor("ones1", [1, P], F32, kind="ExternalInput").ap()
    b4 = nc.dram_tensor("b4", [1, D], F32, kind="ExternalInput").ap()
    iota5 = nc.dram_tensor("iota5", [P, E], F32, kind="ExternalInput").ap()
    y = nc.dram_tensor("y", [S, D], F32, kind="ExternalOutput").ap()
    if debug_taps:
        dbg_emb = nc.dram_tensor("dbg_emb", [P, NT, DA], F32, kind="ExternalOutput").ap()
        dbg_gsc = nc.dram_tensor("dbg_gsc", [P, NT], F32, kind="ExternalOutput").ap()
        dbg_ind = nc.dram_tensor("dbg_ind", [P, NT, E], F32, kind="ExternalOutput").ap()
        dbg_xm = nc.dram_tensor("dbg_xm", [P, D], F32, kind="ExternalOutput").ap()
        dbg_xmt = nc.dram_tensor("dbg_xmt", [P, P], F32R, kind="ExternalOutput").ap()

    with tile.TileContext(nc) as tc:
        with (
            tc.tile_pool(name="singles", bufs=1) as singles,
            tc.tile_pool(name="work", bufs=4) as work,
            tc.tile_pool(name="xmt", bufs=12) as xmt,
            tc.tile_pool(name="gpool", bufs=1) as gpool,
            tc.tile_pool(name="tp_ps", bufs=2, space="PSUM") as tp_ps,
            tc.tile_pool(name="y_ps", bufs=1, space="PSUM") as y_ps,
        ):
            # ids go first, on the scalar-engine HWDGE queue, so the gathers
            # are not stuck behind the weight DMAs on the sync queue
            ids_sb = singles.tile([P, NT], I32)
            nc.scalar.dma_start(out=ids_sb[:], in_=ids[:, :])

            # ---- constants (sync queue, overlaps the gathers) ----
            ones_sb = singles.tile([1, P], F32)
            nc.sync.dma_start(out=ones_sb[:], in_=ones1[:, :])
            b4_sb = singles.tile([1, D], F32)
            nc.sync.dma_start(out=b4_sb[:], in_=b4[:, :])
            ident_sb = singles.tile([P, P], F32R)
            nc.sync.dma_start(out=ident_sb[:], in_=ident[:, :])
            iota5_sb = singles.tile([P, E], F32)
            nc.sync.dma_start(out=iota5_sb[:], in_=iota5[:, :])
            wt_sb = singles.tile([P, NJ, D], F32R)
            for j in range(NJ):
                nc.sync.dma_start(out=wt_sb[:, j, :], in_=wt[:, j, :])

            # Bcast4 = ones^T (x) b4 via 1-partition outer-product matmul
            bc_ps = y_ps.tile([P, D], F32, tag="bcps")
            nc.tensor.matmul(out=bc_ps[:], lhsT=ones_sb[:], rhs=b4_sb[:],
                             start=True, stop=True)
            bcast4 = singles.tile([P, D], F32)
            nc.scalar.activation(out=bcast4[:], in_=bc_ps[:],
                                 func=mybir.ActivationFunctionType.Copy)

            # gather [premasked emb | e(v)] rows per 128-token tile; one
            # SBUF tile per gather so downstream deps are exact
            embs = []
            for t in range(NT):
                emb_t = gpool.tile([P, DA], F32R, tag=f"emb{t}")
                nc.gpsimd.indirect_dma_start(
                    out=emb_t[:],
                    out_offset=None,
                    in_=emb[:, :],
                    in_offset=bass.IndirectOffsetOnAxis(
                        ap=ids_sb[:, t : t + 1], axis=0
                    ),
                )
                embs.append(emb_t)

            if debug_taps:
                for t in range(NT):
                    nc.sync.dma_start(out=dbg_emb[:, t, :], in_=embs[t][:])

            # ---- per 128-token tile ----
            for t in range(NT):
                emb_t = embs[t]
                # one-hot expert indicators from the precomputed e(v) column
                ind_f = work.tile([P, E], F32, tag="indf")
                nc.vector.tensor_scalar(
                    out=ind_f[:], in0=iota5_sb[:], scalar1=emb_t[:, DE : DE + 1].bitcast(F32),
                    scalar2=None, op0=mybir.AluOpType.is_equal,
                )
                ind_i8 = work.tile([P, E], I8, tag="indi")
                nc.scalar.activation(
                    out=ind_i8[:], in_=ind_f[:],
                    func=mybir.ActivationFunctionType.Copy,
                )

                # transpose the 4 K-chunks of the premasked row into one PSUM
                # bank; separate xT tiles per chunk
                tp = tp_ps.tile([P, 4 * P], F32R, tag="tp")
                for k in range(NK):
                    nc.tensor.matmul(
                        out=tp[:, k * P : (k + 1) * P],
                        lhsT=emb_t[:, k * P : (k + 1) * P],
                        rhs=ident_sb[:],
                        is_transpose=True,
                        start=(k == 0), stop=(k == NK - 1),
                    )
                xTs = []
                for k in range(NK):
                    xT_k = xmt.tile([P, P], F32R, tag=f"xmT{k}")
                    nc.scalar.activation(
                        out=xT_k[:], in_=tp[:, k * P : (k + 1) * P],
                        func=mybir.ActivationFunctionType.Copy,
                    )
                    xTs.append(xT_k)
                if debug_taps and t == 0:
                    nc.sync.dma_start(out=dbg_xmt[:, :], in_=xTs[0][:])

                # one PSUM bank per expert; expert e needs chunks 0..ce-1
                banks = []
                for e in range(E):
                    ybank = y_ps.tile([P, D], F32, tag=f"yps{e}")
                    banks.append(ybank)
                for e in range(E):
                    ce = CHUNKS_PER_EXPERT[e]
                    for k in range(ce):
                        j = sum(CHUNKS_PER_EXPERT[:e]) + k
                        nc.tensor.matmul(
                            out=banks[e][:],
                            lhsT=xTs[k][:],
                            rhs=wt_sb[:, j, :],
                            start=(k == 0), stop=(k == ce - 1),
                        )
                # assemble: rows of expert e from bank e, then add bias row
                y_sb = work.tile([P, D], F32, tag="ysb")
                nc.scalar.activation(
                    out=y_sb[:], in_=banks[0][:],
                    func=mybir.ActivationFunctionType.Copy,
                    scale=ind_f[:, 0:1],
                )
                for e in range(1, E):
                    nc.vector.copy_predicated(
                        out=y_sb[:],
                        mask=ind_i8[:, e : e + 1].to_broadcast([P, D]),
                        data=banks[e][:],
                    )
                nc.vector.scalar_tensor_tensor(
                    out=y_sb[:], in0=bcast4[:], scalar=ind_f[:, 4:5],
                    in1=y_sb[:], op0=mybir.AluOpType.mult,
                    op1=mybir.AluOpType.add,
                )
                nc.sync.dma_start(out=y[t * P : (t + 1) * P, :], in_=y_sb[:])

    nc.compile()
    return nc


def prep_core_inputs(input_ids_row, emb_table, gate_table, expert_w, expert_b,
                     aug=None):
    """Host-side layout prep for one core. input_ids_row: (S,) int."""
    ids = np.ascontiguousarray(
        input_ids_row.reshape(NT, P).T.astype(np.int32)
    )  # [P, NT]: ids[p, t] = token t*128+p
    if aug is None:
        aug = build_aug_table(emb_table, gate_table, expert_b)
    # wt[p, j, :] = expert_w[e].T[128k+p, :] = expert_w[e][:, 128k+p] for j=(e,k)
    wt_full = np.transpose(expert_w, (2, 0, 1)).reshape(NK, P, E, D)  # [k,p,e,o]
    cols = []
    for e in range(E):
        for k in range(CHUNKS_PER_EXPERT[e]):
            cols.append(wt_full[k, :, e, :])  # [P, D]
    wt = np.ascontiguousarray(np.stack(cols, axis=1), dtype=np.float32)  # [P,NJ,D]
    for e in range(4):
        j = sum(CHUNKS_PER_EXPERT[:e]) + CHUNKS_PER_EXPERT[e] - 1
        wt[P - 1, j, :] = expert_b[e]
    ident = np.eye(P, dtype=np.float32)
    iota5 = np.ascontiguousarray(
        np.broadcast_to(np.arange(E, dtype=np.float32), (P, E))
    )
    return {
        "ids": ids,
        "emb": aug,
        "wt": wt,
        "ident": ident,
        "iota5": iota5,
        "ones1": np.ones((1, P), np.float32),
        "b4": expert_b[4:5].astype(np.float32),
    }


def build_aug_table(emb_table, gate_table, expert_b):
    """Weights-only preprocessing: premask the embedding row with its gate
    mask, bake the bias-carrier 1.0 into the always-zero feature
    128*(e+1)-1 for e<4, and append the expert index."""
    g = gate_table[:, 0].astype(np.float32) * np.float32(D)
    count = np.ceil(g)
    eidx = np.clip((count // float(D // E)).astype(np.int64), 0, E - 1)
    iota = np.arange(D, dtype=np.float32)
    mask = (iota[None, :] < g[:, None]).astype(np.float32)
    aug = np.zeros((VOCAB, DA), dtype=np.float32)
    aug[:, :D] = emb_table * mask
    for e in range(4):
        aug[eidx == e, P * (e + 1) - 1] = 1.0
    aug[:, DE] = eidx.astype(np.float32)
    return aug


_CACHED_NC = None


def kernel(input_ids, emb_table, gate_table, expert_w, expert_b):
    global _CACHED_NC
    input_ids = np.asarray(input_ids)
    emb_table = np.asarray(emb_table, dtype=np.float32)
    gate_table = np.asarray(gate_table, dtype=np.float32)
    expert_w = np.asarray(expert_w, dtype=np.float32)
    expert_b = np.asarray(expert_b, dtype=np.float32)

    if _CACHED_NC is None:
        _CACHED_NC = build_program()
    nc = _CACHED_NC

    shared = None
    in_maps = []
    for c in range(B):
        m = prep_core_inputs(
            input_ids[c], emb_table, gate_table, expert_w, expert_b,
            aug=None if shared is None else shared["emb"],
        )
        if shared is None:
            shared = m
        else:
            # reuse identical replicated arrays across cores
            for k_ in ("emb", "wt", "ident", "iota5", "ones1", "b4"):
                m[k_] = shared[k_]
        in_maps.append(m)

    trace = bool(int(os.environ.get("BASS_KERNEL_TRACE", "0")))
    res = bass_utils.run_bass_kernel_spmd(
        nc, in_maps, core_ids=list(range(B)), trace=trace
    )
    kernel.last_result = res
    out = np.stack([res.results[c]["y"] for c in range(B)], axis=0)
    return out.astype(np.float32)

